# revision 17
# baseline (speedup 1.0000x reference)
"""Expert-parallel MoE GLU FFN for Trainium2 (8 NeuronCores, Bass/Tile).

Problem: nn_ExpertFFNGroupedMM (E=8 experts, K=2, NTOK=2048, D=1024, DFF=1024,
ALIGN=16). Reference: sort routed rows by expert, pad each expert group to a
multiple of 16, grouped GEMM fc1 (GLU) -> y*silu(gate) -> grouped GEMM fc2,
scatter back to original routed-row order.

Strategy (expert parallelism, per the sharding hint):
  - Host: stable-sort routed rows by expert id, compute per-expert ranks
    (exactly the reference's pad_sorted_idxs math), and build one dense token
    slab per expert, zero-padded to a common static capacity C_pad.
  - Host pre-packs every device input into its exact SBUF layout
    (partition-major), so each input lands in SBUF with a few large
    fully-contiguous DMAs.
  - Device (core e): h = fc1[e] @ x (contraction dim on SBUF partitions ->
    transpose-free), a = y * silu(gate), out = fc2[e] @ a. Matmuls run as
    float32r (full 1 col/cycle PE rate for moving width >= 256, and its
    LDWEIGHTS pipelines under the previous matmul, unlike bf16's).
  - ALL tensors ship bf16 (7.2MB total vs 14.2MB all-fp32): x upcasts to
    f32r per contraction block on the otherwise-idle GpSimd+Scalar engines,
    w1 on Vector (pipelined 2 pairs ahead of the PE), w2 on Vector during
    mid-fc1 slack. Output ships bf16 (~0.2% rounding vs the 2e-2 gate).
  - Head: the measured DMA stream only starts flowing ~8us in (fixed NEFF
    preamble + DMA ring startup), so the issue order is smallest-gating-
    first: w1 pair0 y-block0 (32KB) -> pair0-y rest -> x0 first half ->
    pair0-gate -> x0 second half -> pair1 -> x1 -> pairs 2-7 (stage-pool
    paced) -> w2. First real matmul gates on ~0.3MB instead of ~1.7MB.
  - PE warm-up matmuls on a zeroed tile run through the DMA head so the HAM
    clock governor (1.2 -> 2.4GHz, ~4.6us of sustained PE activity) is at
    full clock when the real stream starts; the warm tile is memset on
    GpSimd whose preamble retires ~1us before Vector's.
  - Tail: dependency-free dummy matmuls after the last fc2 matmul keep the
    PE busy through the out-DMA drain so the NEFF's fixed ~256-semaphore
    teardown (~65 clears/engine) starts at full clock instead of the k=4
    half-clock it measured at otherwise (7.7us -> ~4us).

Measured dead ends (kept for the record): bf16 matmul operands drop the
cadence to 135ns/MM in the 8-core kernel (LDWEIGHTS serializes), mixed
f32r/bf16 operands are rejected by the BIR verifier, walrus --max-sem-num
does not shrink the fixed NEFF semaphore-clear epilogue, --enable-ldw-opt
fails codegen, fp8 fails the 2e-2 gate (~5-9% error), and chunk widths
below 256 run fp32r at quarter rate so the 2x288 chunking is provably the
minimum SPMD capacity for this count distribution.
"""

from functools import lru_cache

import numpy as np
import ml_dtypes

import concourse.bacc as bacc
import concourse.tile as tile
from concourse import mybir
from concourse import bass_utils

E = 8
K = 2
NTOK = 2048
D = 1024
DFF = 1024
ALIGN = 16
N_CORES = 8

DB = D // 128     # 8 contraction blocks for fc1
JB = DFF // 128   # 8 contraction blocks for fc2
NPAIR = DFF // 128  # 8 GLU pairs (y_p, gate_p), each 128 wide

F32 = mybir.dt.float32
F32R = mybir.dt.float32r
BF16 = mybir.dt.bfloat16
NP_BF16 = ml_dtypes.bfloat16

N_WARM_HEAD = 13   # warm-up matmuls (512-col bf16) bridging the DMA head
N_WARM_TAIL = 0  # clock-hold matmuls after the last real matmul


def _plan_chunks(c: int) -> tuple[int, ...]:
    """Split padded capacity c into equal token chunks <=512 (fp32 moving-
    operand max / one PSUM bank), multiples of 8. float32r runs at full PE
    rate only for chunks >=256, which holds whenever c >= 512."""
    n = max(1, -(-c // 512))
    base = -(-c // (8 * n)) * 8
    return (base,) * n


@lru_cache(maxsize=4)
def _build(c_pad: int, chunks: tuple[int, ...]):
    nc = bacc.Bacc("TRN2", target_bir_lowering=False, debug=False)

    # All inputs pre-packed on host to partition-major SBUF layout:
    #   x_pack[p, ci*DB*w + db*w + t]    = x_e.T[db*128+p, ci*w + t]
    #   w1_pack[p, (pr*DB + db)*256 + f] = fc1 pair-interleaved (see pack)
    #   w2_pack[p, jb*D + m]             = fc2[e].T[jb*128+p, m]
    #   out_pack[p, mb*c_pad + t]        = out_e.T[mb*128+p, t]
    # Everything ships bf16 and upcasts on-device (matmuls need f32r, and
    # the BIR verifier rejects mixed 32/16-bit matmul operands).
    x_t = nc.dram_tensor("x_pack", [128, DB * c_pad], BF16,
                         kind="ExternalInput")
    w1_t = nc.dram_tensor("w1_pack", [128, NPAIR * DB * 256], BF16,
                          kind="ExternalInput")
    w2_t = nc.dram_tensor("w2_pack", [128, JB * D], BF16,
                          kind="ExternalInput")
    out_t = nc.dram_tensor("out_pack", [128, DB * c_pad], BF16,
                           kind="ExternalOutput")

    n_chunks = len(chunks)
    with tile.TileContext(nc) as tc:
        with (
            tc.tile_pool(name="xw", bufs=1) as xw,
            tc.tile_pool(name="hpy", bufs=4, space="PSUM") as hpy,
            tc.tile_pool(name="hps", bufs=3, space="PSUM") as hps,
            tc.tile_pool(name="ops", bufs=1, space="PSUM") as ops,
            tc.tile_pool(name="act", bufs=4) as actp,
            tc.tile_pool(name="apool", bufs=1) as apool,
            tc.tile_pool(name="outp", bufs=2) as outp,
            tc.tile_pool(name="wstage", bufs=3) as wstage,
            tc.tile_pool(name="xstage", bufs=1) as xstage,
        ):
            # PE warm-up: dependency-free matmuls on a zeroed tile keep the
            # PE busy through the input-DMA head so the HAM governor is at
            # full clock when the real matmul stream starts. NOTE: GpSimd
            # must stay idle — waking the DSP with real work (casts/memsets)
            # measured the WHOLE chip clock capped at 2.0GHz instead of
            # 2.4GHz (121ns -> 145ns matmul cadence, +15us end to end).
            warm = actp.tile([128, 512], BF16, tag="warm", name="warm")
            nc.vector.memset(warm, 0.0)
            wps = ops.tile([128, 512], F32, tag="o", name="warm_ps")
            for i in range(N_WARM_HEAD):
                nc.tensor.matmul(wps, warm[:, 0:128], warm,
                                 start=(i == 0), stop=(i == N_WARM_HEAD - 1))
            # Touch the Silu table before the first scalar Copy so both
            # activation tables are resident before the real silu stream
            # (the table swap measured 1.5us mid-stream otherwise).
            silu_warm = actp.tile([128, 16], F32, tag="sw", name="silu_warm")
            nc.scalar.activation(
                out=silu_warm, in_=warm[:, 0:16],
                func=mybir.ActivationFunctionType.Silu,
            )

            # All bulk DMAs go through Sync's HWDGE. Issue order = PE
            # dependency order; concurrently-active DMA contexts share the
            # pipe round-robin, so the stream start is set by the cumulative
            # bytes ahead of each prerequisite.
            half = DB * 128
            w1_sb = []
            for p in range(NPAIR):
                t = xw.tile([128, 2 * half], F32R, name=f"w1_{p}")
                w1_sb.append(t)
            x_sb = []
            x_st = []
            for ci, w in enumerate(chunks):
                x_sb.append(xw.tile([128, DB * w], F32R, name=f"x_{ci}"))
                x_st.append(xstage.tile([128, DB * w], BF16, name=f"xs_{ci}"))
            w2_sb = xw.tile([128, JB * D], F32R, name="w2")
            w2_st = xstage.tile([128, JB * D], BF16, name="w2s")

            # bf16 staging for w1: one DMA per pair into a 3-deep pool; the
            # Vector engine upcasts each pair to its resident f32r tile. The
            # pool recycling paces w1 DMA p+3 behind upcast p.
            w1_st = []

            def w1_dma(p, gate=None):
                st = wstage.tile([128, 2 * half], BF16, tag="wst",
                                 name=f"st_{p}")
                if gate is not None:
                    gate(st[:, 0:1])
                nc.sync.dma_start(
                    out=st, in_=w1_t[:, p * 2 * half:(p + 1) * 2 * half])
                w1_st.append(st)

            def w1_upcast(p):
                nc.vector.tensor_copy(w1_sb[p], w1_st[p])

            # Head choreography. The DMA pipe ramps ~30->360GB/s over its
            # first 2us and active transfers share it round-robin, so every
            # extra in-flight context starves the critical chain (measured:
            # with 9 contexts racing, a 0.25MB transfer took 7.5us). Phase A
            # keeps ONLY pair0-y + x chunk0 in flight; each later transfer
            # is released by a tiny WAR-dependency copy that fires when an
            # x0 block finishes upcasting — real data deps the scheduler
            # must honor (issue ORDER alone measured no priority at all).
            # All issues stay on Sync's HWDGE; issuing from the Scalar
            # engine's HWDGE measured the scalar COMPUTE queue blocked
            # behind the issues, starving the x casts and idling the PE
            # mid-clock-ramp — which the HAM governor punished with 13.7us
            # of half clock.
            w0 = chunks[0]
            st0 = wstage.tile([128, 2 * half], BF16, tag="wst", name="st_0")
            nc.sync.dma_start(out=st0[:, 0:half], in_=w1_t[:, 0:half])
            nc.sync.dma_start(out=x_st[0][:, 0:4 * w0],
                              in_=x_t[:, 0:4 * w0])
            nc.sync.dma_start(out=x_st[0][:, 4 * w0:DB * w0],
                              in_=x_t[:, 4 * w0:DB * w0])
            w1_st.append(st0)

            # x upcasts: per contraction block, alternating Vector/Scalar so
            # the cast stream paces at ~2x either engine alone (GpSimd would
            # be the natural third engine but waking it caps the chip clock,
            # and its DSP casts measured 1.6us apiece anyway).
            def x_upcast(ci):
                w = chunks[ci]
                for b in range(DB):
                    dst = x_sb[ci][:, b * w:(b + 1) * w]
                    src = x_st[ci][:, b * w:(b + 1) * w]
                    if b % 2 == 0:
                        nc.vector.tensor_copy(dst, src)
                    else:
                        nc.scalar.copy(dst, src)

            # pair0-y upcast + x chunk0 upcasts (phase A consumers).
            nc.vector.tensor_copy(w1_sb[0][:, 0:half], st0[:, 0:half])
            x_upcast(0)

            def _release(dst_col, b):
                # WAR gate: overwrite one column of the DMA's destination
                # with a copy that depends on x0 block b's upcast; the DMA
                # then cannot start before that cast retires.
                nc.vector.tensor_copy(
                    dst_col, x_sb[0][:, b * w0:b * w0 + 1])

            # Phase B: pair0-gate, pair1 halves, x chunk1 — released as x0
            # upcasts retire, in first-use order.
            _release(st0[:, half:half + 1], 0)
            nc.sync.dma_start(out=st0[:, half:2 * half],
                              in_=w1_t[:, half:2 * half])
            nc.vector.tensor_copy(w1_sb[0][:, half:2 * half],
                                  st0[:, half:2 * half])
            st1 = wstage.tile([128, 2 * half], BF16, tag="wst", name="st_1")
            _release(st1[:, 0:1], 2)
            nc.sync.dma_start(out=st1[:, 0:half],
                              in_=w1_t[:, 2 * half:3 * half])
            nc.vector.tensor_copy(w1_sb[1][:, 0:half], st1[:, 0:half])
            _release(st1[:, half:half + 1], 4)
            nc.sync.dma_start(out=st1[:, half:2 * half],
                              in_=w1_t[:, 3 * half:4 * half])
            nc.vector.tensor_copy(w1_sb[1][:, half:2 * half],
                                  st1[:, half:2 * half])
            w1_st.append(st1)
            for ci in range(1, n_chunks):
                w = chunks[ci]
                _release(x_st[ci][:, 0:1], 6)
                nc.sync.dma_start(
                    out=x_st[ci], in_=x_t[:, ci * DB * w:(ci + 1) * DB * w])
            for p in range(2, NPAIR):
                if p == 2:
                    w1_dma(p, gate=lambda col: _release(col, 6))
                else:
                    w1_dma(p)
            # fc2 weights last (single 2MB transfer); upcast on Vector in
            # four segments during mid-fc1 slack.
            nc.sync.dma_start(out=w2_st, in_=w2_t[:, 0:JB * D])

            # fc1 + gated activation. The first two pairs interleave their
            # chunks ((0,0),(1,0),(0,1),(1,1)) so x chunk1's first use lands
            # later, matching its DMA arrival; later pairs run both chunks
            # back-to-back. Upcasts are emitted where their pair's staging
            # DMA has landed, keeping the Vector queue unblocked. w2's four
            # upcast segments are emitted across the mid-fc1 entries.
            a_sb = {}
            offs = []
            off = 0
            for w in chunks:
                offs.append(off)
                off += w
            seq = []
            ups = []
            x1c = []
            w2c = []
            if n_chunks >= 2:
                seq = [(0, 0), (1, 0), (0, 1), (1, 1)]
                ups = [None, None,
                       2 if NPAIR > 2 else None, 3 if NPAIR > 3 else None]
                x1c = [False, True, False, False]
                w2c = [None, None, None, None]
                for p in range(2, NPAIR):
                    seq.append((p, 0))
                    ups.append(p + 2 if p + 2 < NPAIR else None)
                    x1c.append(False)
                    w2c.append(p - 4 if 4 <= p < 8 else None)
                    for ci in range(1, n_chunks):
                        seq.append((p, ci))
                        ups.append(None)
                        x1c.append(False)
                        w2c.append(None)
            else:
                for p in range(NPAIR):
                    seq.append((p, 0))
                    ups.append(p + 2 if p + 2 < NPAIR else None)
                    x1c.append(False)
                    w2c.append(p - 4 if 4 <= p < 8 else None)
            for (p, ci), up, xc, wc in zip(seq, ups, x1c, w2c):
                w = chunks[ci]
                if up is not None:
                    w1_upcast(up)
                if xc:
                    for cj in range(1, n_chunks):
                        x_upcast(cj)
                if wc is not None:
                    nc.vector.tensor_copy(
                        w2_sb[:, wc * 2 * D:(wc + 1) * 2 * D],
                        w2_st[:, wc * 2 * D:(wc + 1) * 2 * D])
                y_ps = hpy.tile([128, w], F32, tag="y", name=f"y_{ci}_{p}")
                g_ps = hps.tile([128, w], F32, tag="g", name=f"g_{ci}_{p}")
                for b in range(DB):
                    rhs = x_sb[ci][:, b * w:b * w + w]
                    lhs = w1_sb[p][:, b * 128:b * 128 + 128]
                    nc.tensor.matmul(y_ps, lhs, rhs,
                                     start=(b == 0), stop=(b == DB - 1))
                for b in range(DB):
                    rhs = x_sb[ci][:, b * w:b * w + w]
                    lhs = w1_sb[p][:, half + b * 128:half + b * 128 + 128]
                    nc.tensor.matmul(g_ps, lhs, rhs,
                                     start=(b == 0), stop=(b == DB - 1))
                silu = actp.tile([128, w], F32, tag="silu", name=f"s_{ci}_{p}")
                nc.scalar.activation(
                    out=silu, in_=g_ps,
                    func=mybir.ActivationFunctionType.Silu,
                )
                a = apool.tile([128, w], F32R, tag=f"a{ci}_{p}",
                               name=f"a_{ci}_{p}")
                nc.vector.tensor_mul(a, y_ps, silu)
                a_sb[ci, p] = a

            # fc2, per chunk; each result block copies to SBUF (DVE, casting
            # to bf16) and ships immediately with its own DMA, so the tail
            # after the last matmul is one small copy + one 73KB transfer.
            o_last = None
            for ci, w in enumerate(chunks):
                off = offs[ci]
                o_sb = outp.tile([128, DB * w], BF16, tag="osb", name=f"ob_{ci}")
                o_last = o_sb
                for mb in range(DB):
                    # Reuse the fc1 y-slots (4 bufs): during fc2 they are
                    # free, and deep PSUM pipelining hides the DVE copy
                    # latency after every group.
                    o_ps = hpy.tile([128, w], F32, tag="y", name=f"o_{ci}_{mb}")
                    for b in range(JB):
                        nc.tensor.matmul(
                            o_ps,
                            w2_sb[:, b * D + mb * 128:b * D + (mb + 1) * 128],
                            a_sb[ci, b],
                            start=(b == 0), stop=(b == JB - 1),
                        )
                    nc.vector.tensor_copy(o_sb[:, mb * w:(mb + 1) * w], o_ps)
                    nc.sync.dma_start(
                        out=out_t[:, mb * c_pad + off:mb * c_pad + off + w],
                        in_=o_sb[:, mb * w:(mb + 1) * w],
                    )

            # Clock hold: matmuls keep the PE (and the HAM clock) busy
            # through the out-DMA drain so the fixed NEFF semaphore teardown
            # starts at full clock. The moving operand reads the final
            # output tile so the Tile scheduler cannot hoist these into the
            # supply-limited head (dep-free dummies measured hoisted there,
            # costing ~2us of early-stream PE slots).
            if N_WARM_TAIL and o_last is not None:
                wps2 = ops.tile([128, 512], F32, tag="o", name="tail_ps")
                for i in range(N_WARM_TAIL):
                    nc.tensor.matmul(wps2, warm[:, 0:128], o_last[:, 0:512],
                                     start=(i == 0),
                                     stop=(i == N_WARM_TAIL - 1))

    nc.compile()
    return nc


def _route(indices: np.ndarray, counts: np.ndarray):
    """Reference's sort/rank math: stable sort by expert, rank within group."""
    n = indices.size
    flat = indices.reshape(-1).astype(np.int64)
    order = np.argsort(flat, kind="stable")
    exp_sorted = flat[order]
    counts64 = counts.astype(np.int64)
    u_start = np.cumsum(counts64) - counts64
    rank = np.arange(n, dtype=np.int64) - u_start[exp_sorted]
    return order, exp_sorted, rank, counts64


def _pack_inputs(x, fc1_weight, fc2_weight, e, sel, r, tok_sel, c_pad, chunks):
    """Build core e's packed inputs (see _build docstring for layouts)."""
    xe = np.zeros((c_pad, D), np.float32)
    xe[r] = x[tok_sel]
    # (c_pad, D) -> xT (D, c_pad) -> (db, 128, ci, w) -> (128, ci, db, w)
    n_chunks = len(chunks)
    w = chunks[0]
    x_pack = np.ascontiguousarray(
        xe.T.reshape(DB, 128, n_chunks, w)
        .transpose(1, 2, 0, 3)
        .reshape(128, DB * c_pad)
    ).astype(NP_BF16)
    # fc1[e]: (2*DFF, D); pair-interleave rows: (y_p, gate_p) adjacent.
    # w1_pack[p, (pr*DB+db)*256 + s*128 + f] = fc1[e][s*DFF + pr*128 + f,
    #                                                 db*128 + p]
    w1 = fc1_weight[e].reshape(2, NPAIR, 128, DB, 128)
    # dims: (s, pr, f, db, p) -> (p, pr, s, db, f): per pair, the y columns
    # (DB*128) then the gate columns (DB*128). Shipped bf16 (upcast to f32r
    # on-device); ~0.4% weight rounding vs the 2e-2 gate.
    w1_pack = np.ascontiguousarray(
        w1.transpose(4, 1, 0, 3, 2).reshape(128, NPAIR * DB * 256)
    ).astype(NP_BF16)
    # fc2[e]: (D, DFF); w2_pack[p, jb*D + m] = fc2[e][m, jb*128 + p]
    w2 = fc2_weight[e].reshape(D, JB, 128)  # (m, jb, p)
    w2_pack = np.ascontiguousarray(
        w2.transpose(2, 1, 0).reshape(128, JB * D)
    ).astype(NP_BF16)
    return {"x_pack": x_pack, "w1_pack": w1_pack, "w2_pack": w2_pack}


# Above this capacity the resident-SBUF layout risks overflowing (x slab +
# weights + activations approach the ~208KB/partition limit); fall back to
# exact host compute beyond it.
MAX_C_PAD = 960


def _numpy_fallback(x, fc1_weight, fc2_weight, order, exp_sorted, rank):
    """Exact reference math on host for pathologically skewed expert counts."""
    n = order.size
    out = np.zeros((n, D), np.float32)
    tok = order // K
    for e in range(E):
        sel = exp_sorted == e
        xe = x[tok[sel]]
        h = xe @ fc1_weight[e].T
        y, gate = h[:, :DFF], h[:, DFF:]
        a = y * (gate / (1.0 + np.exp(-gate)))
        out[order[sel]] = a @ fc2_weight[e].T
    return out


def _prepare(x, fc1_weight, fc2_weight, indices, counts):
    """Route on host, build/fetch the compiled kernel, pack per-core inputs."""
    x = np.ascontiguousarray(np.asarray(x, dtype=np.float32))
    fc1_weight = np.asarray(fc1_weight, dtype=np.float32)
    fc2_weight = np.asarray(fc2_weight, dtype=np.float32)
    indices = np.asarray(indices)
    counts = np.asarray(counts)

    order, exp_sorted, rank, counts64 = _route(indices, counts)
    tok = order // K

    padded = ((counts64 + ALIGN - 1) // ALIGN) * ALIGN
    c = max(int(padded.max()), ALIGN)
    chunks = _plan_chunks(c)
    c_pad = int(sum(chunks))
    if c_pad > MAX_C_PAD:
        return None, None, (order, exp_sorted, rank,
                            (x, fc1_weight, fc2_weight))

    nc = _build(c_pad, chunks)

    # With consistent inputs (counts == bincount(indices)) every rank lies in
    # [0, c_pad); guard anyway so inconsistent counts can't index out of range.
    valid = (rank >= 0) & (rank < c_pad)

    in_maps = []
    sels = []
    for e in range(E):
        sel = (exp_sorted == e) & valid
        sels.append(sel)
        in_maps.append(
            _pack_inputs(x, fc1_weight, fc2_weight, e, sel, rank[sel],
                         tok[sel], c_pad, chunks)
        )
    return nc, in_maps, (order, sels, rank, c_pad, indices.size)


def _unpack_outputs(results, meta):
    order, sels, rank, c_pad, n = meta
    out = np.zeros((n, D), np.float32)
    for e in range(E):
        sel = sels[e]
        # out_pack (128, DB*c_pad) -> outT (D, c_pad) -> (c_pad, D)
        op = np.asarray(results[e]["out_pack"]).astype(np.float32)
        op = op.reshape(128, DB, c_pad)
        oe = op.transpose(1, 0, 2).reshape(D, c_pad)
        out[order[sel]] = oe.T[rank[sel]]
    return out


# run_bass_kernel_spmd's axon path rebuilds and re-traces a fresh jax.jit
# wrapper on every call (~2s warm overhead for a ~60us device kernel). Build
# the same shard_map executable once per compiled kernel and cache it; any
# failure falls back to the stock path.
_RUNNERS = {}


def _get_runner(nc):
    r = _RUNNERS.get(id(nc))
    if r is not None:
        return r
    import jax
    from concourse import bass2jax

    bass2jax.install_neuronx_cc_hook()
    assert nc.dbg_addr is None
    pid_name = nc.partition_id_tensor.name if nc.partition_id_tensor else None
    in_names, out_names, out_avals = [], [], []
    for alloc in nc.m.functions[0].allocations:
        if not isinstance(alloc, mybir.MemoryLocationSet):
            continue
        name = alloc.memorylocations[0].name
        if alloc.kind == "ExternalInput":
            if name != pid_name:
                in_names.append(name)
        elif alloc.kind == "ExternalOutput":
            out_avals.append(jax.core.ShapedArray(
                tuple(alloc.tensor_shape), mybir.dt.np(alloc.dtype)))
            out_names.append(name)
    n_params, n_outs = len(in_names), len(out_names)
    all_names = in_names + out_names
    if pid_name is not None:
        all_names.append(pid_name)
    all_names = tuple(all_names)

    def _body(*args):
        operands = list(args)
        if pid_name is not None:
            operands.append(bass2jax.partition_id_tensor())
        return tuple(bass2jax._bass_exec_p.bind(
            *operands,
            out_avals=tuple(out_avals), in_names=all_names,
            out_names=tuple(out_names), lowering_input_output_aliases=(),
            sim_require_finite=True, sim_require_nnan=True, nc=nc))

    devices = jax.devices()[:N_CORES]
    mesh = bass2jax.Mesh(np.asarray(devices), ("core",))
    sharded = jax.jit(
        bass2jax.shard_map(
            _body, mesh=mesh,
            in_specs=(bass2jax.PartitionSpec("core"),) * (n_params + n_outs),
            out_specs=(bass2jax.PartitionSpec("core"),) * n_outs,
            check_rep=False),
        donate_argnums=tuple(range(n_params, n_params + n_outs)),
        keep_unused=True)
    r = (sharded, in_names, out_names, out_avals)
    _RUNNERS[id(nc)] = r
    return r


def _run_cached(nc, in_maps):
    sharded, in_names, out_names, out_avals = _get_runner(nc)
    concat_in = [
        np.concatenate([np.asarray(m[n]) for m in in_maps], axis=0)
        for n in in_names
    ]
    concat_zeros = [
        np.zeros((N_CORES * a.shape[0], *a.shape[1:]), a.dtype)
        for a in out_avals
    ]
    out_arrs = sharded(*concat_in, *concat_zeros)
    return [
        {n: np.asarray(out_arrs[i]).reshape(N_CORES, *out_avals[i].shape)[c]
         for i, n in enumerate(out_names)}
        for c in range(N_CORES)
    ]


def kernel(x, fc1_weight, fc2_weight, indices, counts):
    nc, in_maps, meta = _prepare(x, fc1_weight, fc2_weight, indices, counts)
    if nc is None:
        order, exp_sorted, rank, arrs = meta
        return _numpy_fallback(arrs[0], arrs[1], arrs[2],
                               order, exp_sorted, rank)
    try:
        results = _run_cached(nc, in_maps)
    except Exception:
        _RUNNERS.pop(id(nc), None)
        results = bass_utils.run_bass_kernel_spmd(
            nc, in_maps, core_ids=list(range(N_CORES))).results
    return _unpack_outputs(results, meta)
